# revision 1
# baseline (speedup 1.0000x reference)
"""TRN2 Bass kernel for multi-head self-attention with RoPE (causal).

Problem: B=4, S=2048, D=768, H=12 heads of dk=64, fp32 in/out.

Sharding: 8 cores = 4 batches x 2 head-groups of 6 heads. Each core computes
QKV projections for its 6 heads, RoPE, causal flash attention, and a
partial output projection; the host sums the two partials per batch.

Numerics: split-bf16 (hi+lo) 3-term matmuls for the Q/K projections and
for Q.K^T scores (the softmax is hyper-argmax: scaled score std ~600,
top-2 gap ~150, so bf16/tf32 rounding would flip winners). Row max must
be EXACT: subsampled estimates overflow bf16 p (gap spacing ~150 >> the
88-e-fold bf16 window). V/AV/O-proj in plain bf16.

Structure (v3, PE-density rewrite):
  - V projection upfront (dense warm-up stream) into v_ext with an
    appended ones column per head, so the AV matmul's 65th output row is
    the softmax denominator for free.
  - Main loop: per pair p, per 512-col x chunk c: Q/K proj matmuls + rope
    (spread over DVE/GpSimd/ACT/DMA), interleaved with attention S/A
    groups of already-projected heads so the PE never idles and DVE's
    attention-phase overhang (exact row max) is absorbed into the proj
    windows.
  - S-group (h, g=4 q-tiles): scores (banded 2-term matmul + k_lo
    correction), mask, exact row max, exp on ACT, XBAR transpose to pts.
  - A-group: column-major AV (one accumulation group per (h,g), k-block
    matmuls of width <=512), flash merge = single per-column rescale of
    the AV psum at the k=1024 boundary, then normalize via
    reciprocal_approx_fast + partition_broadcast into av_all.
  - O-projection tail.
"""

import sys

sys.path.insert(0, "/opt/trn_rl_repo")

from contextlib import ExitStack

import ml_dtypes
import numpy as np

import concourse.bass as bass
import concourse.tile as tile
from concourse import bacc, mybir
from concourse.bass_utils import run_bass_kernel_spmd

F32 = mybir.dt.float32
BF16 = mybir.dt.bfloat16
bf16 = ml_dtypes.bfloat16

B, D, H, DK = 4, 768, 12, 64
NHC = 6          # heads per core
NPAIR = 3        # head pairs per core
DSUB = 6         # d_in subtiles of 128
CPC = NHC * DK   # 384 head-dims per core
CH = 1024        # score chunk along k


def _build(S=2048, trace_label="", debug_out=False):
    NQT = S // 128       # 16 q-tiles
    NG = NQT // 4        # 4 q-groups per head (512 q cols each)
    nc = bacc.Bacc("TRN2", target_bir_lowering=False, debug=False, num_devices=8)

    def din(name, shape, dt):
        return nc.dram_tensor(name, shape, dt, kind="ExternalInput").ap()

    xh_d = din("xh", [128, DSUB, S], BF16)
    xl_d = din("xl", [128, DSUB, S], BF16)
    wqh_d = din("wqh", [128, DSUB, CPC], BF16)
    wql_d = din("wql", [128, DSUB, CPC], BF16)
    wkh_d = din("wkh", [128, DSUB, CPC], BF16)
    wkl_d = din("wkl", [128, DSUB, CPC], BF16)
    wv_d = din("wvT", [128, DSUB, CPC], BF16)
    wo_d = din("woT", [128, NPAIR, D], BF16)
    cos_d = din("cos_t", [128, S], F32)
    sin_d = din("sin_t", [128, S], F32)
    mask_d = din("mask", [128, 128], F32)
    out_d = nc.dram_tensor("out", [S, D], F32, kind="ExternalOutput").ap()
    if debug_out:
        rec_d = nc.dram_tensor("rec_dbg", [NHC, S], F32,
                               kind="ExternalOutput").ap()
        avd_d = nc.dram_tensor("av_dbg", [128, NPAIR, S], BF16,
                               kind="ExternalOutput").ap()
        q_d = nc.dram_tensor("q_dbg", [128, NHC, S], BF16,
                             kind="ExternalOutput").ap()
        k_d = nc.dram_tensor("k_dbg", [128, NHC, S], BF16,
                             kind="ExternalOutput").ap()
        psb_d = nc.dram_tensor("psb_dbg", [4, 128, CH], BF16,
                               kind="ExternalOutput").ap()
        pts_d = nc.dram_tensor("pts_dbg", [128, 4, NQT, 128], BF16,
                               kind="ExternalOutput").ap()
        avp_d = nc.dram_tensor("avp_dbg", [65, 512], F32,
                               kind="ExternalOutput").ap()

    SUB = mybir.AluOpType.subtract
    MIN = mybir.AluOpType.min

    with tile.TileContext(nc) as tc, ExitStack() as ctx:
        # ---------- persistent SBUF ----------
        pers = ctx.enter_context(tc.tile_pool(name="pers", bufs=1))

        def load(pool, dr, name):
            t = pool.tile(list(dr.shape), dr.dtype, tag=f"L{name}")
            nc.sync.dma_start(t[:], dr[:])
            return t

        mask = load(pers, mask_d, "mask")
        cos_t = load(pers, cos_d, "cos")
        sin_t = load(pers, sin_d, "sin")

        # band layouts (all matmul operands at base partition 0):
        # q_hl: band0 = q_hi, band1 = q_lo; k_hh: k_hi in both bands;
        # k_l: k_lo on partitions 0:64
        q_hl = pers.tile([128, NHC, S], BF16, tag="q_hl")
        k_hh = pers.tile([128, NHC, S], BF16, tag="k_hh")
        k_l = pers.tile([64, NHC, S], BF16, tag="k_l")
        # v with an appended ones column per head: AV matmul row 64 = sum(p)
        v_ext = pers.tile([128, NQT, NHC, 65], BF16, tag="v_ext")
        av_all = pers.tile([128, NPAIR, S], BF16, tag="av_all")

        # ---------- V projection upfront ----------
        with tc.tile_pool(name="vw", bufs=1) as vw, \
             tc.tile_pool(name="vx", bufs=2) as vx, \
             tc.tile_pool(name="pv", bufs=2, space="PSUM") as pvs:
            wv = load(vw, wv_d, "wv")
            nc.gpsimd.memset(v_ext[:, :, :, 64:65], 1.0)
            for sc_i in range(S // 512):
                ssl = bass.ts(sc_i, 512)
                xv = vx.tile([128, DSUB, 512], BF16, tag="xv")
                nc.sync.dma_start(xv[:], xh_d[:, :, ssl])
                for st4 in range(4):
                    st = sc_i * 4 + st4
                    psv = pvs.tile([128, 512], F32, tag="pv")
                    for t in range(DSUB):
                        nc.tensor.matmul(
                            psv[:, 0:CPC],
                            xv[:, t, bass.ts(st4, 128)], wv[:, t, :],
                            start=(t == 0), stop=(t == DSUB - 1),
                        )
                    nc.scalar.copy(out=v_ext[:, st, :, 0:64], in_=psv[:, 0:CPC])

        # ---------- main: Q/K proj pairs interleaved with attention ----------
        with tc.tile_pool(name="wsl", bufs=2) as wsl, \
             tc.tile_pool(name="bx", bufs=2) as bx, \
             tc.tile_pool(name="rwork", bufs=2) as rwk, \
             tc.tile_pool(name="pqk", bufs=2, space="PSUM") as pps, \
             tc.tile_pool(name="scps", bufs=2, space="PSUM") as scps, \
             tc.tile_pool(name="avps", bufs=2, space="PSUM") as avps, \
             tc.tile_pool(name="psbp", bufs=2) as psbp, \
             tc.tile_pool(name="ptsp", bufs=2) as ptsp, \
             tc.tile_pool(name="stats", bufs=4) as stp, \
             tc.tile_pool(name="gdp", bufs=2) as gdp, \
             tc.tile_pool(name="b0p", bufs=2) as b0p, \
             tc.tile_pool(name="nwork", bufs=1) as nwk:

            def load_wslices(p):
                ws = {}
                for nm, dr in (("qh", wqh_d), ("ql", wql_d),
                               ("kh", wkh_d), ("kl", wkl_d)):
                    t = wsl.tile([128, DSUB, 128], BF16, tag=f"w{nm}")
                    nc.sync.dma_start(t[:], dr[:, :, bass.ts(p, 128)])
                    ws[nm] = t
                return ws

            def proj_chunk(p, c, ws):
                ssl = bass.ts(c, 512)
                xc = bx.tile([128, 2, DSUB, 512], BF16, tag="xc")
                nc.sync.dma_start(xc[:, 0], xh_d[:, :, ssl])
                nc.sync.dma_start(xc[:, 1], xl_d[:, :, ssl])
                # q then k through one rotating 1-bank psum tile each
                for qk, (w_hi, w_lo) in enumerate(
                    ((ws["qh"], ws["ql"]), (ws["kh"], ws["kl"]))
                ):
                    pqk = pps.tile([128, 512], F32, tag="pp")
                    n = 0
                    for t in range(DSUB):
                        for lh, xi in ((w_hi, 0), (w_hi, 1), (w_lo, 0)):
                            nc.tensor.matmul(
                                pqk[:],
                                lh[:, t, :],
                                xc[:, xi, t, :],
                                start=(n == 0), stop=(n == 3 * DSUB - 1),
                            )
                            n += 1
                    # rope (2 heads stacked on partitions)
                    f32c = rwk.tile([128, 512], F32, tag="f32c")
                    nc.scalar.copy(out=f32c[:], in_=pqk[:])
                    swp = rwk.tile([128, 512], F32, tag="swp")
                    for a in range(2):
                        nc.sync.dma_start(
                            swp[64 * a:64 * a + 32, :],
                            f32c[64 * a + 32:64 * a + 64, :],
                        )
                        nc.sync.dma_start(
                            swp[64 * a + 32:64 * a + 64, :],
                            f32c[64 * a:64 * a + 32, :],
                        )
                    # rotate in place: f32c *= cos (after swaps read it),
                    # swp = swp*sin + f32c
                    nc.vector.tensor_mul(f32c[:], f32c[:], cos_t[:, ssl])
                    nc.gpsimd.tensor_mul(swp[:], swp[:], sin_t[:, ssl])
                    nc.gpsimd.tensor_add(swp[:], swp[:], f32c[:])
                    for sub in range(2):
                        hh = 2 * p + sub
                        band = swp[64 * sub:64 * sub + 64, :]
                        if sub == 0:
                            b0 = band
                        else:
                            b0t = b0p.tile([64, 512], F32, tag="b0t")
                            nc.sync.dma_start(b0t[:], band)
                            b0 = b0t[:]
                        if qk == 0:
                            nc.scalar.copy(
                                out=q_hl[0:64, hh, ssl], in_=b0)
                            nc.vector.tensor_tensor(
                                q_hl[64:128, hh, ssl], b0,
                                q_hl[0:64, hh, ssl], SUB,
                            )
                        else:
                            nc.scalar.copy(
                                out=k_hh[0:64, hh, ssl], in_=b0)
                            nc.vector.tensor_tensor(
                                k_l[0:64, hh, ssl], b0,
                                k_hh[0:64, hh, ssl], SUB,
                            )
                            nc.sync.dma_start(
                                k_hh[64:128, hh, ssl],
                                k_hh[0:64, hh, ssl],
                            )

            pts_tiles = {}
            gd_tiles = {}

            def s_group(gi, h, g):
                # pts[p, qtl, j, c]: p^T of q-tile (4g+qtl), k-block j;
                # transpose DMA dst [:, qtl, j0:j0+nblk, :] is contiguous
                pts = ptsp.tile([128, 4, NQT, 128], BF16, tag="pts")
                pts_tiles[gi] = pts
                if g >= 2:
                    gd = gdp.tile([1, 512], BF16, tag="gdelta")
                    gd_tiles[gi] = gd
                for qtl in range(4):
                    qt = 4 * g + qtl
                    nk = (qt + 1) * 128
                    qsl = bass.ts(qt, 128)
                    chunks = [(0, min(CH, nk))]
                    if nk > CH:
                        chunks.append((CH, nk - CH))
                    nm1 = None
                    for ci, (k0, nkc) in enumerate(chunks):
                        sc = scps.tile([128, CH], F32, tag="sc")
                        for n0 in range(0, nkc, 512):
                            nn = min(512, nkc - n0)
                            ksl = bass.ds(k0 + n0, nn)
                            nc.tensor.matmul(
                                sc[:, bass.ds(n0, nn)],
                                q_hl[:, h, qsl], k_hh[:, h, ksl],
                                start=True, stop=False,
                            )
                            nc.tensor.matmul(
                                sc[:, bass.ds(n0, nn)],
                                q_hl[0:64, h, qsl], k_l[:, h, ksl],
                                start=False, stop=True,
                            )
                        if ci == len(chunks) - 1:
                            doff = nk - 128 - k0
                            nc.vector.tensor_add(
                                sc[:, bass.ds(doff, 128)],
                                sc[:, bass.ds(doff, 128)], mask[:],
                            )
                        mx = stp.tile([128, 1], F32, tag="mx")
                        nc.vector.tensor_reduce(
                            mx[:], sc[:, 0:nkc], mybir.AxisListType.X,
                            mybir.AluOpType.max,
                        )
                        nm8 = stp.tile([128, 1], F32, tag=f"nm{ci}")
                        nc.vector.tensor_scalar_mul(nm8[:], mx[:], -0.125)
                        if ci == 0:
                            nm1 = nm8
                            bias = nm8
                        else:
                            nmf = stp.tile([128, 1], F32, tag="nmf")
                            nc.vector.tensor_tensor(
                                nmf[:], nm8[:], nm1[:], MIN)
                            delta = stp.tile([128, 1], BF16, tag="delta")
                            nc.vector.tensor_tensor(
                                delta[:], nmf[:], nm1[:], SUB)
                            nc.sync.dma_start(
                                gd_tiles[gi][0:1, bass.ts(qtl, 128)],
                                delta[:, 0:1],
                            )
                            bias = nmf
                        psb = psbp.tile([128, CH], BF16, tag="psb")
                        nc.scalar.activation(
                            psb[:, 0:nkc], sc[:, 0:nkc],
                            mybir.ActivationFunctionType.Exp,
                            bias=bias[:, 0:1], scale=0.125,
                        )
                        # transpose in 256-col pieces, alternating dispatch
                        # rings, so AV never waits on one deep DMA queue
                        j0 = k0 // 128
                        for tp0 in range(0, nkc, 256):
                            tpn = min(256, nkc - tp0)
                            eng = nc.sync if (tp0 // 256) % 2 == 0 \
                                else nc.scalar
                            eng.dma_start_transpose(
                                pts[:, qtl,
                                    j0 + tp0 // 128:
                                    j0 + (tp0 + tpn) // 128, :],
                                psb[:, bass.ds(tp0, tpn)],
                            )
                        if debug_out and h == 0 and g == 0:
                            nc.sync.dma_start(
                                psb_d[qtl, :, 0:nkc], psb[:, 0:nkc])

            def a_group(gi, h, g):
                pts = pts_tiles.pop(gi)
                av = avps.tile([65, 512], F32, tag="av")
                jmax = 4 * g + 3
                ph1 = list(range(min(8, jmax + 1)))
                ph2 = list(range(8, jmax + 1))

                def emit(js, first_starts, skip):
                    for idx, j in enumerate(js):
                        q0 = max(j - 4 * g, 0)
                        nc.tensor.matmul(
                            av[:, bass.ds(q0 * 128, (4 - q0) * 128)],
                            v_ext[:, j, h, :],
                            pts[:, q0:4, j, :],
                            start=(first_starts and idx == 0),
                            stop=(idx == len(js) - 1),
                            skip_group_check=skip,
                        )

                emit(ph1, True, False)
                if debug_out and h == 0 and g == 0:
                    nc.sync.dma_start(pts_d[:], pts[:])
                    avc = nwk.tile([65, 512], F32, tag="avc_dbg")
                    nc.scalar.copy(out=avc[:], in_=av[:, :])
                    nc.sync.dma_start(avp_d[:], avc[:])
                if ph2:
                    abr = nwk.tile([1, 512], BF16, tag="abr")
                    nc.scalar.activation(
                        abr[:], gd_tiles.pop(gi)[:],
                        mybir.ActivationFunctionType.Exp,
                    )
                    ab = nwk.tile([65, 512], BF16, tag="ab")
                    nc.gpsimd.partition_broadcast(ab[:], abr[0:1, :])
                    nc.vector.tensor_mul(av[:, :], av[:, :], ab[:])
                    emit(ph2, False, True)
                # normalize into av_all (O-proj lhsT layout)
                dro = nwk.tile([1, 512], F32, tag="dro")
                nc.vector.tensor_copy(dro[:], av[64:65, :])
                rec = nwk.tile([1, 512], F32, tag="rec")
                nc.vector.reciprocal_approx_fast(out=rec[:], in_=dro[:])
                if debug_out:
                    nc.sync.dma_start(
                        rec_d[h:h + 1, bass.ts(g, 512)], rec[:])
                recb = nwk.tile([64, 512], F32, tag="recb")
                nc.gpsimd.partition_broadcast(recb[:], rec[0:1, :])
                hl, pr = h % 2, h // 2
                nc.vector.tensor_mul(
                    av_all[64 * hl:64 * hl + 64, pr, bass.ts(g, 512)],
                    av[0:64, :], recb[:],
                )

            # ---- emission schedule: proj windows absorb attention ----
            sorder = [(2 * p + i, g)
                      for p in range(NPAIR) for g in range(NG) for i in (0, 1)]
            si = 0

            def emit_sa(budget, p, c):
                nonlocal si
                while si < len(sorder) and budget > 0:
                    h, g = sorder[si]
                    if p is not None and not (
                        h // 2 < p or (h // 2 == p and g < c)
                    ):
                        return
                    s_group(si, h, g)
                    if si > 0:
                        a_group(si - 1, *sorder[si - 1])
                    si += 1
                    budget -= 1

            ws_cur = load_wslices(0)
            for p in range(NPAIR):
                ws_next = load_wslices(p + 1) if p < NPAIR - 1 else None
                for c in range(S // 512):
                    proj_chunk(p, c, ws_cur)
                    emit_sa(2, p, c)
                ws_cur = ws_next
            emit_sa(len(sorder), None, None)
            a_group(len(sorder) - 1, *sorder[-1])
            if debug_out:
                nc.sync.dma_start(avd_d[:], av_all[:])
                nc.sync.dma_start(q_d[:], q_hl[:])
                nc.sync.dma_start(k_d[:], k_hh[:])

        # ---------- output projection ----------
        with tc.tile_pool(name="ops", bufs=2, space="PSUM") as ops, \
             tc.tile_pool(name="wop", bufs=1) as wop, \
             tc.tile_pool(name="owork", bufs=3) as owk:
            wo = load(wop, wo_d, "wo")
            for st in range(NQT):
                po = ops.tile([128, 2, 512], F32, tag="po")
                for half in range(2):
                    for p in range(NPAIR):
                        nc.tensor.matmul(
                            po[:, half, 0:384],
                            av_all[:, p, bass.ts(st, 128)],
                            wo[:, p, bass.ts(half, 384)],
                            start=(p == 0), stop=(p == NPAIR - 1),
                        )
                osb = owk.tile([128, D], F32, tag="osb")
                nc.scalar.copy(out=osb[:, 0:384], in_=po[:, 0, 0:384])
                nc.scalar.copy(out=osb[:, 384:768], in_=po[:, 1, 0:384])
                nc.sync.dma_start(out_d[bass.ts(st, 128), :], osb[:])

    nc.compile()
    return nc


def _rope_perm():
    p = np.zeros(DK, dtype=np.int64)
    for i in range(DK // 2):
        p[i] = 2 * i
        p[i + 32] = 2 * i + 1
    return p


def _split(a):
    hi = a.astype(bf16)
    lo = (a.astype(np.float32) - hi.astype(np.float32)).astype(bf16)
    return hi, lo


def _tile_din(a):
    # [768, F] -> [128, 6, F]
    return np.ascontiguousarray(a.reshape(DSUB, 128, -1).transpose(1, 0, 2))


def make_inputs(x, wq, wk, wv, wo, S):
    """Host-side prep: returns list of 8 in_maps (core = 2*b + g)."""
    perm = _rope_perm()
    pos = np.arange(S, dtype=np.float64)
    inv = 10000.0 ** (-2.0 * np.arange(DK // 2, dtype=np.float64) / DK)
    ang = pos[:, None] * inv[None, :]
    cosv = np.cos(ang).astype(np.float32).T  # [32, S]
    sinv = np.sin(ang).astype(np.float32).T
    cos_t = np.tile(cosv, (4, 1)).astype(np.float32)            # [128, S]
    sin_t = np.tile(
        np.concatenate([-sinv, sinv], axis=0), (2, 1)
    ).astype(np.float32)                                        # [128, S]
    mask = np.triu(np.full((128, 128), -1e9, np.float32), 1)

    maps = []
    for b in range(B):
        xT = np.ascontiguousarray(x[b].T.astype(np.float32))  # [768, S]
        xh, xl = _split(xT)
        xh_t, xl_t = _tile_din(xh), _tile_din(xl)
        for g in range(2):
            hs = slice(g * CPC, (g + 1) * CPC)
            wqc = wq[hs].astype(np.float32).copy()
            wkc = wk[hs].astype(np.float32).copy()
            for arr in (wqc, wkc):
                for i in range(NHC):
                    blk = arr[i * DK:(i + 1) * DK].copy()
                    arr[i * DK:(i + 1) * DK] = blk[perm]
            wqh, wql = _split(wqc.T)  # [768, 384]
            wkh, wkl = _split(wkc.T)
            wvT = wv[hs].astype(np.float32).T.astype(bf16)
            woT = wo[:, hs].astype(np.float32).T.astype(bf16)  # [384, 768]
            maps.append({
                "xh": xh_t, "xl": xl_t,
                "wqh": _tile_din(wqh), "wql": _tile_din(wql),
                "wkh": _tile_din(wkh), "wkl": _tile_din(wkl),
                "wvT": _tile_din(wvT),
                "woT": np.ascontiguousarray(
                    woT.reshape(NPAIR, 128, D).transpose(1, 0, 2)),
                "cos_t": cos_t, "sin_t": sin_t, "mask": mask,
            })
    return maps


_PROG = {}


def _prog(S):
    if S not in _PROG:
        _PROG[S] = _build(S)
    return _PROG[S]


def kernel(x, wq, wk, wv, wo, S=2048, trace=False):
    x = np.asarray(x, np.float32)
    nc = _prog(S)
    maps = make_inputs(x, np.asarray(wq), np.asarray(wk), np.asarray(wv),
                       np.asarray(wo), S)
    res = run_bass_kernel_spmd(nc, maps, list(range(8)), trace=trace)
    outs = []
    for b in range(B):
        outs.append(res.results[2 * b]["out"] + res.results[2 * b + 1]["out"])
    out = np.stack(outs)
    if trace:
        kernel.last_exec_time_ns = res.exec_time_ns
        kernel.last_results = res
    return out



# revision 16
# speedup vs baseline: 1.2031x; 1.2031x over previous
"""TRN2 Bass kernel for multi-head self-attention with RoPE (causal).

Problem: B=4, S=2048, D=768, H=12 heads of dk=64, fp32 in/out.

Sharding: 8 cores = 4 batches x 2 head-groups of 6 heads. Each core computes
QKV projections for its 6 heads, RoPE, causal attention, and a partial
output projection; the host sums the two partials per batch.

v4: TRANSPOSED-SCORES rewrite. The v3 kernel spent ~525us of Sync/Scalar
dispatch + 16 DMA queues on XBAR-transposing P [q,k] -> [k,q] for the AV
matmul, starving the PE (51% busy, p-state never ramped). v4 computes the
precise scores directly transposed, S^T[k,q] = K Q^T, so exp(S^T) IS the
AV rhs and no transpose of score-sized data ever happens:

  - cheap-max pass [q,k]: one 2-term matmul per 512-block
    ((qhi+qlo)@khi via band layout), DVE row-max (mask fused into the
    last block via tensor_tensor_reduce with a right-aligned [0|tri]
    mask), per-128-q-tile maxes -> negate -> bf16 -> one [128,128] XBAR
    transpose per (head, q-group) -> row [1,512] of -mhat.
    Empirically |rowmax_cheap - rowmax_precise| <= 7 raw-score units;
    the exp window tolerates ~60, and any consistent bias cancels in the
    normalization, so the cheap max only has to bound, not match.
  - precise S^T per k-tile j (same 3 terms as v3 = qhi@khi + qlo@khi +
    qhi@klo): mm1 = k_hl[khi|klo bands] x q_hh[qhi|qhi], mm2 =
    k_ho[khi;ones][65] x ql_b[qlo;-mhat][65] -- the 65th contract row
    adds -mhat to every score for free, solving the "per-column bias"
    problem of the transposed layout.
  - mask_t on diagonal tiles, ACT exp (scale=1/8, no bias) -> pts bf16,
    AV accumulates v_ext[128,65-with-ones] x pts into [65,512] psum;
    row 64 = softmax denominator for free; reciprocal+broadcast
    normalize into av_all.
  - PE p-state care: TRN2's tensor clock ramps 0.65->1.2->2.4GHz with
    ~3us of continuous busy; every idle gap halves throughput for the
    next 3us. A weave scheduler interleaves (proj bursts) x (cheap of
    slot t+1) x (S^T/AV of slot t) at matmul granularity so the PE
    stream stays dense and all DVE/ACT post-processing hides under it.
"""

import sys

sys.path.insert(0, "/opt/trn_rl_repo")

from collections import deque
from contextlib import ExitStack

import ml_dtypes
import numpy as np

import concourse.bass as bass
import concourse.tile as tile
from concourse import bacc, mybir
from concourse.bass_utils import run_bass_kernel_spmd

F32 = mybir.dt.float32
BF16 = mybir.dt.bfloat16
bf16 = ml_dtypes.bfloat16

B, D, H, DK = 4, 768, 12, 64
NHC = 6          # heads per core
NPAIR = 3        # head pairs per core
DSUB = 6         # d_in subtiles of 128
CPC = NHC * DK   # 384 head-dims per core

SUB = mybir.AluOpType.subtract
ADD = mybir.AluOpType.add
MAX = mybir.AluOpType.max
EXP = mybir.ActivationFunctionType.Exp


def _build(S=2048, trace_label=""):
    NQT = S // 128       # 16 q-tiles
    NG = NQT // 4        # 4 q-groups per head (512 q cols each)
    nc = bacc.Bacc("TRN2", target_bir_lowering=False, debug=False,
                   num_devices=8)

    def din(name, shape, dt):
        return nc.dram_tensor(name, shape, dt, kind="ExternalInput").ap()

    xh_d = din("xh", [128, DSUB, S], BF16)
    xl_d = din("xl", [128, DSUB, S], BF16)
    wqh_d = din("wqh", [128, DSUB, CPC], BF16)
    wql_d = din("wql", [128, DSUB, CPC], BF16)
    wkh_d = din("wkh", [128, DSUB, CPC], BF16)
    wkl_d = din("wkl", [128, DSUB, CPC], BF16)
    wv_d = din("wvT", [128, DSUB, CPC], BF16)
    wo_d = din("woT", [128, NPAIR, D], BF16)
    cos_d = din("cos_t", [128, S], F32)
    sin_d = din("sin_t", [128, S], F32)
    m512_d = din("mask512", [128, 512], F32)   # [zeros(384) | triu128]
    maskt_d = din("mask_t", [128, 128], F32)   # tril(-1e9, -1): [k,q] diag
    out_d = nc.dram_tensor("out", [S, D], F32, kind="ExternalOutput").ap()

    with tile.TileContext(nc) as tc, ExitStack() as ctx:
        # ---------- persistent SBUF ----------
        pers = ctx.enter_context(tc.tile_pool(name="pers", bufs=1))

        def load(pool, dr, name):
            t = pool.tile(list(dr.shape), dr.dtype, tag=f"L{name}")
            nc.sync.dma_start(t[:], dr[:])
            return t

        mask512 = load(pers, m512_d, "m512")
        mask_t = load(pers, maskt_d, "maskt")
        cos_t = load(pers, cos_d, "cos")
        sin_t = load(pers, sin_d, "sin")

        # v with an appended ones column per head: AV matmul row 64 = sum(p)
        v_ext = pers.tile([128, NQT, NHC, 65], BF16, tag="v_ext")
        av_all = pers.tile([128, NPAIR, S], BF16, tag="av_all")

        # ---------- V projection upfront ----------
        with tc.tile_pool(name="vw", bufs=1) as vw, \
             tc.tile_pool(name="vx", bufs=2) as vx, \
             tc.tile_pool(name="pv", bufs=2, space="PSUM") as pvs:
            wv = load(vw, wv_d, "wv")
            nc.gpsimd.memset(v_ext[:, :, :, 64:65], 1.0)

            def load_xv(sc_i):
                xv = vx.tile([128, DSUB, 512], BF16, tag="xv")
                nc.sync.dma_start(xv[:], xh_d[:, :, bass.ts(sc_i, 512)])
                return xv

            xv_cur = load_xv(0)
            for sc_i in range(S // 512):
                xv = xv_cur
                if sc_i < S // 512 - 1:
                    xv_cur = load_xv(sc_i + 1)
                for st4 in range(4):
                    st = sc_i * 4 + st4
                    psv = pvs.tile([128, 512], F32, tag="pv")
                    for t in range(DSUB):
                        nc.tensor.matmul(
                            psv[:, 0:CPC],
                            xv[:, t, bass.ts(st4, 128)], wv[:, t, :],
                            start=(t == 0), stop=(t == DSUB - 1),
                        )
                    nc.scalar.copy(out=v_ext[:, st, :, 0:64], in_=psv[:, 0:CPC])

        # ---------- main loop ----------
        with tc.tile_pool(name="wsl", bufs=2) as wsl, \
             tc.tile_pool(name="bx", bufs=2) as bx, \
             tc.tile_pool(name="rwork", bufs=2) as rwk, \
             tc.tile_pool(name="b0p", bufs=2) as b0p, \
             tc.tile_pool(name="qkp", bufs=2) as qkp, \
             tc.tile_pool(name="scp", bufs=4, space="PSUM") as scp, \
             tc.tile_pool(name="avp", bufs=2, space="PSUM") as avp, \
             tc.tile_pool(name="pqk", bufs=2, space="PSUM") as pps, \
             tc.tile_pool(name="ptsp", bufs=4) as ptsp, \
             tc.tile_pool(name="stgp", bufs=2) as stgp, \
             tc.tile_pool(name="stats", bufs=4) as stp, \
             tc.tile_pool(name="nwork", bufs=2) as nwk:

            def load_wslices(p):
                ws = {}
                for nm, dr in (("qh", wqh_d), ("ql", wql_d),
                               ("kh", wkh_d), ("kl", wkl_d)):
                    t = wsl.tile([128, DSUB, 128], BF16, tag=f"w{nm}")
                    nc.sync.dma_start(t[:], dr[:, :, bass.ts(p, 128)])
                    ws[nm] = t
                return ws

            def load_xc(c_seq):
                # c_seq = global 512-col chunk index (shared by all pairs
                # of the same c); reload per (p, c) to keep pool rotation
                # simple -- the load is prefetched one chunk ahead.
                xc = bx.tile([128, 2, DSUB, 512], BF16, tag="xc")
                ssl = bass.ts(c_seq % (S // 512), 512)
                nc.sync.dma_start(xc[:, 0], xh_d[:, :, ssl])
                nc.sync.dma_start(xc[:, 1], xl_d[:, :, ssl])
                return xc

            def pair_tiles(p):
                # per-pair q/k band layouts, auto-rotated (bufs=2):
                #  q_hl: [qhi|qlo]   (cheap-pass lhsT)
                #  q_hh: [qhi|qhi]   (S^T mm1 rhs)
                #  ql_b: [qlo; -mhat](S^T mm2 rhs, 65 partitions)
                #  k_hl: [khi|klo]   (S^T mm1 lhsT)
                #  k_hh: [khi|khi]   (cheap-pass rhs)
                #  k_ho: [khi; ones] (S^T mm2 lhsT, 65 partitions)
                t = {}
                for nm in ("q_hl", "q_hh", "k_hl", "k_hh"):
                    t[nm] = qkp.tile([128, 2, S], BF16, tag=nm, name=nm)
                for nm in ("ql_b", "k_ho"):
                    t[nm] = qkp.tile([65, 2, S], BF16, tag=nm, name=nm)
                return t

            def ones_memset(tl):
                nc.gpsimd.memset(tl["k_ho"][64:65, :, :], 1.0)

            def proj_gen(p, c, ws, tl, xc):
                """Q/K projection + rope for pair p, seq chunk c (xc
                preloaded). Yields between matmul bursts."""
                ssl = bass.ts(c, 512)
                for qk, (w_hi, w_lo) in enumerate(
                    ((ws["qh"], ws["ql"]), (ws["kh"], ws["kl"]))
                ):
                    pqk = pps.tile([128, 512], F32, tag="pp")
                    n = 0
                    for t in range(DSUB):
                        for lh, xi in ((w_hi, 0), (w_hi, 1), (w_lo, 0)):
                            nc.tensor.matmul(
                                pqk[:], lh[:, t, :], xc[:, xi, t, :],
                                start=(n == 0), stop=(n == 3 * DSUB - 1),
                            )
                            n += 1
                            if n % 6 == 0 and n < 18:
                                yield 6 * 512
                    yield 6 * 512
                    # rope (2 heads stacked on partitions)
                    f32c = rwk.tile([128, 512], F32, tag="f32c")
                    nc.scalar.copy(out=f32c[:], in_=pqk[:])
                    swp = rwk.tile([128, 512], F32, tag="swp")
                    for a in range(2):
                        nc.sync.dma_start(
                            swp[64 * a:64 * a + 32, :],
                            f32c[64 * a + 32:64 * a + 64, :],
                        )
                        nc.sync.dma_start(
                            swp[64 * a + 32:64 * a + 64, :],
                            f32c[64 * a:64 * a + 32, :],
                        )
                    nc.gpsimd.tensor_mul(f32c[:], f32c[:], cos_t[:, ssl])
                    nc.gpsimd.tensor_mul(swp[:], swp[:], sin_t[:, ssl])
                    nc.gpsimd.tensor_add(swp[:], swp[:], f32c[:])
                    for sub in range(2):
                        hh = sub  # head slot within pair
                        band = swp[64 * sub:64 * sub + 64, :]
                        if sub == 0:
                            b0 = band
                        else:
                            b0t = b0p.tile([64, 512], F32, tag="b0t")
                            nc.sync.dma_start(b0t[:], band)
                            b0 = b0t[:]
                        if qk == 0:
                            q_hl, q_hh, ql_b = tl["q_hl"], tl["q_hh"], tl["ql_b"]
                            nc.scalar.copy(out=q_hl[0:64, hh, ssl], in_=b0)
                            nc.vector.tensor_tensor(
                                q_hl[64:128, hh, ssl], b0,
                                q_hl[0:64, hh, ssl], SUB,
                            )
                            nc.sync.dma_start(
                                q_hh[0:64, hh, ssl], q_hl[0:64, hh, ssl])
                            nc.sync.dma_start(
                                q_hh[64:128, hh, ssl], q_hl[0:64, hh, ssl])
                            nc.sync.dma_start(
                                ql_b[0:64, hh, ssl], q_hl[64:128, hh, ssl])
                        else:
                            k_hl, k_hh, k_ho = tl["k_hl"], tl["k_hh"], tl["k_ho"]
                            nc.scalar.copy(out=k_hl[0:64, hh, ssl], in_=b0)
                            nc.vector.tensor_tensor(
                                k_hl[64:128, hh, ssl], b0,
                                k_hl[0:64, hh, ssl], SUB,
                            )
                            nc.sync.dma_start(
                                k_hh[0:64, hh, ssl], k_hl[0:64, hh, ssl])
                            nc.sync.dma_start(
                                k_hh[64:128, hh, ssl], k_hl[0:64, hh, ssl])
                            nc.sync.dma_start(
                                k_ho[0:64, hh, ssl], k_hl[0:64, hh, ssl])

            def cheap_gen(hh, g, tl):
                """Cheap 2-term [q,k] pass for slot (head-in-pair hh, group g):
                row maxes -> -mhat bf16 row written to ql_b[64, hh, g*512:]."""
                q_hl, k_hh, ql_b = tl["q_hl"], tl["k_hh"], tl["ql_b"]
                stage = stgp.tile([128, 128], BF16, tag="stage")
                nc.gpsimd.memset(stage[:, 4:128], 0.0)
                for qtl in range(4):
                    qt = 4 * g + qtl
                    nk = (qt + 1) * 128
                    nblk = (nk + 511) // 512
                    qsl = bass.ts(qt, 128)
                    mxp = stp.tile([128, 4], F32, tag="mxp")
                    for b in range(nblk):
                        k0 = 512 * b
                        nn = min(512, nk - k0)
                        sc = scp.tile([128, 512], F32, tag="sc")
                        nc.tensor.matmul(
                            sc[:, 0:nn],
                            q_hl[:, hh, qsl], k_hh[:, hh, bass.ds(k0, nn)],
                            start=True, stop=True,
                        )
                        if b == nblk - 1:
                            # causal mask on the diagonal 128 cols
                            # (tensor_tensor_reduce would fuse this but
                            # crashes TRN2 hw)
                            nc.vector.tensor_tensor(
                                sc[:, bass.ds(nn - 128, 128)],
                                sc[:, bass.ds(nn - 128, 128)],
                                mask512[:, 384:512], ADD,
                            )
                        nc.vector.tensor_reduce(
                            mxp[:, b:b + 1], sc[:, 0:nn],
                            mybir.AxisListType.X, MAX,
                        )
                        yield nn
                    # combine partials, negate, round to bf16
                    nc.vector.tensor_reduce(
                        stage[:, qtl:qtl + 1], mxp[:, 0:nblk],
                        mybir.AxisListType.X, MAX, negate=True,
                    )
                # transpose [q,qtl] -> [qtl,q]; write -mhat row into ql_b
                outt = stgp.tile([128, 128], BF16, tag="outt")
                nc.sync.dma_start_transpose(outt[:], stage[:])
                for qtl in range(4):
                    qt = 4 * g + qtl
                    nc.sync.dma_start(
                        ql_b[64:65, hh, bass.ts(qt, 128)],
                        outt[qtl:qtl + 1, 0:128],
                    )

            def stav_gen(hh, g, tl, h_abs):
                """Precise S^T + exp + AV for slot (hh, g). Yields between
                matmul groups. st leads av by 2 k-tiles."""
                k_hl, k_ho = tl["k_hl"], tl["k_ho"]
                q_hh, ql_b = tl["q_hh"], tl["ql_b"]
                jmax = 4 * g + 3
                av = avp.tile([65, 512], F32, tag="av")
                gq0 = g * 512
                pend = deque()  # (j, pts_tile, q0, nq)

                def av_mm(j, pts_t, q0, nq):
                    nc.tensor.matmul(
                        av[:, bass.ds(q0, nq)],
                        v_ext[:, j, h_abs % NHC, :], pts_t[:, 0:nq],
                        start=(j == 0), stop=(j == jmax),
                        skip_group_check=True,
                    )

                for j in range(jmax + 1):
                    q0 = max(0, (j - 4 * g) * 128)
                    nq = 512 - q0
                    jsl = bass.ts(j, 128)
                    st = scp.tile([128, 512], F32, tag="sc")
                    nc.tensor.matmul(
                        st[:, bass.ds(q0, nq)],
                        k_hl[:, hh, jsl],
                        q_hh[:, hh, bass.ds(gq0 + q0, nq)],
                        start=True, stop=False,
                    )
                    nc.tensor.matmul(
                        st[:, bass.ds(q0, nq)],
                        k_ho[0:65, hh, jsl],
                        ql_b[0:65, hh, bass.ds(gq0 + q0, nq)],
                        start=False, stop=True,
                    )
                    if j >= 4 * g:  # diagonal tile: causal mask in [k,q]
                        nc.vector.tensor_tensor(
                            st[:, bass.ds(q0, 128)],
                            st[:, bass.ds(q0, 128)], mask_t[:], ADD,
                        )
                    pts_t = ptsp.tile([128, 512], BF16, tag="pts")
                    nc.scalar.activation(
                        pts_t[:, 0:nq], st[:, bass.ds(q0, nq)],
                        EXP, scale=0.125,
                    )
                    pend.append((j, pts_t, q0, nq))
                    yield 2 * nq
                    if len(pend) > 2:
                        av_mm(*pend.popleft())
                        yield 512
                while pend:
                    av_mm(*pend.popleft())
                    yield 512
                # normalize: row 64 holds the denominator
                dro = nwk.tile([1, 512], F32, tag="dro")
                nc.vector.tensor_copy(dro[:], av[64:65, :])
                rec = nwk.tile([1, 512], F32, tag="rec")
                nc.vector.reciprocal_approx_fast(out=rec[:], in_=dro[:])
                recb = nwk.tile([64, 512], F32, tag="recb")
                nc.gpsimd.partition_broadcast(recb[:], rec[0:1, :])
                hl, pr = h_abs % 2, h_abs // 2
                nc.vector.tensor_mul(
                    av_all[64 * hl:64 * hl + 64, pr, bass.ts(g, 512)],
                    av[0:64, :], recb[:],
                )

            # ---- weave scheduler ----
            # slots in order: for p, for g, for head-in-pair
            attnq = deque()   # active attention generators [(kind, gen)]
            state = {"attn_cols": 0, "proj_cols": 0}

            def pull(gen):
                try:
                    cols = next(gen[1])
                    state["attn_cols"] += cols
                    return True
                except StopIteration:
                    try:
                        attnq.remove(gen)
                    except ValueError:
                        pass
                    return False

            def pump_attn(target_ratio=1.9, max_units=10**9):
                """Advance attention gens: primary = head of queue, weave
                with the first independent 'cheap' gen behind it."""
                units = 0
                while attnq and units < max_units and (
                    state["attn_cols"] < target_ratio * state["proj_cols"]
                    or target_ratio < 0
                ):
                    primary = attnq[0]
                    if not pull(primary):
                        continue
                    units += 1
                    sec = None
                    for gq in list(attnq)[1:]:
                        if gq[0] == "cheap":
                            sec = gq
                            break
                    if sec is not None:
                        pull(sec)
                        units += 1

            ws_cur = load_wslices(0)
            tl_cur = pair_tiles(0)
            ones_memset(tl_cur)
            xc_cur = load_xc(0)
            NCH = S // 512
            for p in range(NPAIR):
                ws_next = load_wslices(p + 1) if p < NPAIR - 1 else None
                tl_next = pair_tiles(p + 1) if p < NPAIR - 1 else None
                for c in range(NCH):
                    if c == NCH - 1 and tl_next is not None:
                        ones_memset(tl_next)
                    xc = xc_cur
                    if NCH * p + c < NPAIR * NCH - 1:
                        xc_cur = load_xc(c + 1)
                    for _cols in proj_gen(p, c, ws_cur, tl_cur, xc):
                        state["proj_cols"] += _cols
                        pump_attn(max_units=3)
                    g = c
                    for hh in range(2):
                        attnq.append(("cheap", cheap_gen(hh, g, tl_cur)))
                    for hh in range(2):
                        attnq.append(
                            ("stav", stav_gen(hh, g, tl_cur, 2 * p + hh)))
                    pump_attn()
                ws_cur, tl_cur = ws_next, tl_next
            # flush remaining attention
            while attnq:
                pump_attn(target_ratio=-1)

        # ---------- output projection ----------
        with tc.tile_pool(name="ops", bufs=2, space="PSUM") as ops, \
             tc.tile_pool(name="wop", bufs=1) as wop, \
             tc.tile_pool(name="owork", bufs=3) as owk:
            wo = load(wop, wo_d, "wo")
            for st in range(NQT):
                po = ops.tile([128, 2, 512], F32, tag="po")
                for half in range(2):
                    for p in range(NPAIR):
                        nc.tensor.matmul(
                            po[:, half, 0:384],
                            av_all[:, p, bass.ts(st, 128)],
                            wo[:, p, bass.ts(half, 384)],
                            start=(p == 0), stop=(p == NPAIR - 1),
                        )
                osb = owk.tile([128, D], F32, tag="osb")
                nc.scalar.copy(out=osb[:, 0:384], in_=po[:, 0, 0:384])
                nc.scalar.copy(out=osb[:, 384:768], in_=po[:, 1, 0:384])
                nc.sync.dma_start(out_d[bass.ts(st, 128), :], osb[:])

    nc.compile()
    return nc


def _rope_perm():
    p = np.zeros(DK, dtype=np.int64)
    for i in range(DK // 2):
        p[i] = 2 * i
        p[i + 32] = 2 * i + 1
    return p


def _split(a):
    hi = a.astype(bf16)
    lo = (a.astype(np.float32) - hi.astype(np.float32)).astype(bf16)
    return hi, lo


def _tile_din(a):
    # [768, F] -> [128, 6, F]
    return np.ascontiguousarray(a.reshape(DSUB, 128, -1).transpose(1, 0, 2))


def make_inputs(x, wq, wk, wv, wo, S):
    """Host-side prep: returns list of 8 in_maps (core = 2*b + g)."""
    perm = _rope_perm()
    pos = np.arange(S, dtype=np.float64)
    inv = 10000.0 ** (-2.0 * np.arange(DK // 2, dtype=np.float64) / DK)
    ang = pos[:, None] * inv[None, :]
    cosv = np.cos(ang).astype(np.float32).T  # [32, S]
    sinv = np.sin(ang).astype(np.float32).T
    cos_t = np.tile(cosv, (4, 1)).astype(np.float32)            # [128, S]
    sin_t = np.tile(
        np.concatenate([-sinv, sinv], axis=0), (2, 1)
    ).astype(np.float32)                                        # [128, S]
    # [zeros(384) | triu(-1e9, 1)]: right-aligned causal mask for the
    # cheap-max pass's last block ([q,k] orientation)
    mask512 = np.zeros((128, 512), np.float32)
    mask512[:, 384:] = np.triu(np.full((128, 128), -1e9, np.float32), 1)
    # [k,q] diagonal-tile mask: invalid k > q
    mask_t = np.tril(np.full((128, 128), -1e9, np.float32), -1)

    maps = []
    for b in range(B):
        xT = np.ascontiguousarray(x[b].T.astype(np.float32))  # [768, S]
        xh, xl = _split(xT)
        xh_t, xl_t = _tile_din(xh), _tile_din(xl)
        for g in range(2):
            hs = slice(g * CPC, (g + 1) * CPC)
            wqc = wq[hs].astype(np.float32).copy()
            wkc = wk[hs].astype(np.float32).copy()
            for arr in (wqc, wkc):
                for i in range(NHC):
                    blk = arr[i * DK:(i + 1) * DK].copy()
                    arr[i * DK:(i + 1) * DK] = blk[perm]
            wqh, wql = _split(wqc.T)  # [768, 384]
            wkh, wkl = _split(wkc.T)
            wvT = wv[hs].astype(np.float32).T.astype(bf16)
            woT = wo[:, hs].astype(np.float32).T.astype(bf16)  # [384, 768]
            maps.append({
                "xh": xh_t, "xl": xl_t,
                "wqh": _tile_din(wqh), "wql": _tile_din(wql),
                "wkh": _tile_din(wkh), "wkl": _tile_din(wkl),
                "wvT": _tile_din(wvT),
                "woT": np.ascontiguousarray(
                    woT.reshape(NPAIR, 128, D).transpose(1, 0, 2)),
                "cos_t": cos_t, "sin_t": sin_t,
                "mask512": mask512, "mask_t": mask_t,
            })
    return maps


_PROG = {}


def _prog(S):
    if S not in _PROG:
        _PROG[S] = _build(S)
    return _PROG[S]


def kernel(x, wq, wk, wv, wo, S=2048, trace=False):
    x = np.asarray(x, np.float32)
    nc = _prog(S)
    maps = make_inputs(x, np.asarray(wq), np.asarray(wk), np.asarray(wv),
                       np.asarray(wo), S)
    res = run_bass_kernel_spmd(nc, maps, list(range(8)), trace=trace)
    outs = []
    for b in range(B):
        outs.append(res.results[2 * b]["out"] + res.results[2 * b + 1]["out"])
    out = np.stack(outs)
    if trace:
        kernel.last_exec_time_ns = res.exec_time_ns
        kernel.last_results = res
    return out


# revision 18
# speedup vs baseline: 1.4146x; 1.1758x over previous
"""TRN2 Bass kernel for multi-head self-attention with RoPE (causal).

Problem: B=4, S=2048, D=768, H=12 heads of dk=64, fp32 in/out.

Sharding: 8 cores = 4 batches x 2 head-groups of 6 heads. Each core computes
QKV projections for its 6 heads, RoPE, causal attention, and a partial
output projection; the host sums the two partials per batch.

v4: TRANSPOSED-SCORES rewrite. The v3 kernel spent ~525us of Sync/Scalar
dispatch + 16 DMA queues on XBAR-transposing P [q,k] -> [k,q] for the AV
matmul, starving the PE (51% busy, p-state never ramped). v4 computes the
precise scores directly transposed, S^T[k,q] = K Q^T, so exp(S^T) IS the
AV rhs and no transpose of score-sized data ever happens:

  - cheap-max pass [q,k]: one 2-term matmul per 512-block
    ((qhi+qlo)@khi via band layout), DVE row-max (mask fused into the
    last block via tensor_tensor_reduce with a right-aligned [0|tri]
    mask), per-128-q-tile maxes -> negate -> bf16 -> one [128,128] XBAR
    transpose per (head, q-group) -> row [1,512] of -mhat.
    Empirically |rowmax_cheap - rowmax_precise| <= 7 raw-score units;
    the exp window tolerates ~60, and any consistent bias cancels in the
    normalization, so the cheap max only has to bound, not match.
  - precise S^T per k-tile j (same 3 terms as v3 = qhi@khi + qlo@khi +
    qhi@klo): mm1 = k_hl[khi|klo bands] x q_hh[qhi|qhi], mm2 =
    k_ho[khi;ones][65] x ql_b[qlo;-mhat][65] -- the 65th contract row
    adds -mhat to every score for free, solving the "per-column bias"
    problem of the transposed layout.
  - mask_t on diagonal tiles, ACT exp (scale=1/8, no bias) -> pts bf16,
    AV accumulates v_ext[128,65-with-ones] x pts into [65,512] psum;
    row 64 = softmax denominator for free; reciprocal+broadcast
    normalize into av_all.
  - PE p-state care: TRN2's tensor clock ramps 0.65->1.2->2.4GHz with
    ~3us of continuous busy; every idle gap halves throughput for the
    next 3us. A weave scheduler interleaves (proj bursts) x (cheap of
    slot t+1) x (S^T/AV of slot t) at matmul granularity so the PE
    stream stays dense and all DVE/ACT post-processing hides under it.
"""

import sys

sys.path.insert(0, "/opt/trn_rl_repo")

from collections import deque
from contextlib import ExitStack

import ml_dtypes
import numpy as np

import concourse.bass as bass
import concourse.tile as tile
from concourse import bacc, mybir
from concourse.bass_utils import run_bass_kernel_spmd

F32 = mybir.dt.float32
BF16 = mybir.dt.bfloat16
bf16 = ml_dtypes.bfloat16

B, D, H, DK = 4, 768, 12, 64
NHC = 6          # heads per core
NPAIR = 3        # head pairs per core
DSUB = 6         # d_in subtiles of 128
CPC = NHC * DK   # 384 head-dims per core

SUB = mybir.AluOpType.subtract
ADD = mybir.AluOpType.add
MAX = mybir.AluOpType.max
EXP = mybir.ActivationFunctionType.Exp


def _build(S=2048, trace_label=""):
    NQT = S // 128       # 16 q-tiles
    NG = NQT // 4        # 4 q-groups per head (512 q cols each)
    nc = bacc.Bacc("TRN2", target_bir_lowering=False, debug=False,
                   num_devices=8)

    def din(name, shape, dt):
        return nc.dram_tensor(name, shape, dt, kind="ExternalInput").ap()

    xh_d = din("xh", [128, DSUB, S], BF16)
    xl_d = din("xl", [128, DSUB, S], BF16)
    wqh_d = din("wqh", [128, DSUB, CPC], BF16)
    wql_d = din("wql", [128, DSUB, CPC], BF16)
    wkh_d = din("wkh", [128, DSUB, CPC], BF16)
    wkl_d = din("wkl", [128, DSUB, CPC], BF16)
    wv_d = din("wvT", [128, DSUB, CPC], BF16)
    wo_d = din("woT", [128, NPAIR, D], BF16)
    cos_d = din("cos_t", [128, S], F32)
    sin_d = din("sin_t", [128, S], F32)
    m512_d = din("mask512", [128, 512], F32)   # [zeros(384) | triu128]
    maskt_d = din("mask_t", [128, 128], F32)   # tril(-1e9, -1): [k,q] diag
    out_d = nc.dram_tensor("out", [S, D], F32, kind="ExternalOutput").ap()

    with tile.TileContext(nc) as tc, ExitStack() as ctx:
        # ---------- persistent SBUF ----------
        pers = ctx.enter_context(tc.tile_pool(name="pers", bufs=1))

        def load(pool, dr, name):
            t = pool.tile(list(dr.shape), dr.dtype, tag=f"L{name}")
            nc.sync.dma_start(t[:], dr[:])
            return t

        mask512 = load(pers, m512_d, "m512")
        mask_t = load(pers, maskt_d, "maskt")
        cos_t = load(pers, cos_d, "cos")
        sin_t = load(pers, sin_d, "sin")

        # v with an appended ones column per head: AV matmul row 64 = sum(p)
        v_ext = pers.tile([128, NQT, NHC, 65], BF16, tag="v_ext")
        av_all = pers.tile([128, NPAIR, S], BF16, tag="av_all")

        # ---------- V projection upfront ----------
        with tc.tile_pool(name="vw", bufs=1) as vw, \
             tc.tile_pool(name="vx", bufs=2) as vx, \
             tc.tile_pool(name="pv", bufs=2, space="PSUM") as pvs:
            wv = load(vw, wv_d, "wv")
            nc.gpsimd.memset(v_ext[:, :, :, 64:65], 1.0)

            def load_xv(sc_i):
                xv = vx.tile([128, DSUB, 512], BF16, tag="xv")
                nc.sync.dma_start(xv[:], xh_d[:, :, bass.ts(sc_i, 512)])
                return xv

            xv_cur = load_xv(0)
            for sc_i in range(S // 512):
                xv = xv_cur
                if sc_i < S // 512 - 1:
                    xv_cur = load_xv(sc_i + 1)
                for st4 in range(4):
                    st = sc_i * 4 + st4
                    psv = pvs.tile([128, 512], F32, tag="pv")
                    for t in range(DSUB):
                        nc.tensor.matmul(
                            psv[:, 0:CPC],
                            xv[:, t, bass.ts(st4, 128)], wv[:, t, :],
                            start=(t == 0), stop=(t == DSUB - 1),
                        )
                    nc.scalar.copy(out=v_ext[:, st, :, 0:64], in_=psv[:, 0:CPC])

        # ---------- main loop ----------
        with tc.tile_pool(name="wsl", bufs=2) as wsl, \
             tc.tile_pool(name="bx", bufs=2) as bx, \
             tc.tile_pool(name="rwork", bufs=2) as rwk, \
             tc.tile_pool(name="b0p", bufs=2) as b0p, \
             tc.tile_pool(name="qkp", bufs=2) as qkp, \
             tc.tile_pool(name="scp", bufs=4, space="PSUM") as scp, \
             tc.tile_pool(name="avp", bufs=2, space="PSUM") as avp, \
             tc.tile_pool(name="pqk", bufs=2, space="PSUM") as pps, \
             tc.tile_pool(name="ptsp", bufs=4) as ptsp, \
             tc.tile_pool(name="stgp", bufs=2) as stgp, \
             tc.tile_pool(name="stats", bufs=4) as stp, \
             tc.tile_pool(name="nwork", bufs=2) as nwk:

            def load_wslices(p):
                ws = {}
                for nm, dr in (("qh", wqh_d), ("ql", wql_d),
                               ("kh", wkh_d), ("kl", wkl_d)):
                    t = wsl.tile([128, DSUB, 128], BF16, tag=f"w{nm}")
                    nc.sync.dma_start(t[:], dr[:, :, bass.ts(p, 128)])
                    ws[nm] = t
                return ws

            def load_xc(c_seq):
                # c_seq = global 512-col chunk index (shared by all pairs
                # of the same c); reload per (p, c) to keep pool rotation
                # simple -- the load is prefetched one chunk ahead.
                xc = bx.tile([128, 2, DSUB, 512], BF16, tag="xc")
                ssl = bass.ts(c_seq % (S // 512), 512)
                nc.sync.dma_start(xc[:, 0], xh_d[:, :, ssl])
                nc.sync.dma_start(xc[:, 1], xl_d[:, :, ssl])
                return xc

            def pair_tiles(p):
                # per-pair q/k band layouts, auto-rotated (bufs=2):
                #  q_hl: [qhi|qlo]   (cheap-pass lhsT)
                #  q_hh: [qhi|qhi]   (S^T mm1 rhs)
                #  ql_b: [qlo; -mhat](S^T mm2 rhs, 65 partitions)
                #  k_hl: [khi|klo]   (S^T mm1 lhsT)
                #  k_hh: [khi|khi]   (cheap-pass rhs)
                #  k_ho: [khi; ones] (S^T mm2 lhsT, 65 partitions)
                t = {}
                for nm in ("q_hl", "q_hh", "k_hl", "k_hh"):
                    t[nm] = qkp.tile([128, 2, S], BF16, tag=nm, name=nm)
                for nm in ("ql_b", "k_ho"):
                    t[nm] = qkp.tile([65, 2, S], BF16, tag=nm, name=nm)
                return t

            def ones_memset(tl):
                nc.gpsimd.memset(tl["k_ho"][64:65, :, :], 1.0)

            def proj_gen(p, c, ws, tl, xc):
                """Q/K projection + rope for pair p, seq chunk c (xc
                preloaded). Yields between matmul bursts."""
                ssl = bass.ts(c, 512)
                for qk, (w_hi, w_lo) in enumerate(
                    ((ws["qh"], ws["ql"]), (ws["kh"], ws["kl"]))
                ):
                    pqk = pps.tile([128, 512], F32, tag="pp")
                    n = 0
                    for t in range(DSUB):
                        for lh, xi in ((w_hi, 0), (w_hi, 1), (w_lo, 0)):
                            nc.tensor.matmul(
                                pqk[:], lh[:, t, :], xc[:, xi, t, :],
                                start=(n == 0), stop=(n == 3 * DSUB - 1),
                            )
                            n += 1
                            if n % 6 == 0 and n < 18:
                                yield 6 * 512
                    yield 6 * 512
                    # rope (2 heads stacked on partitions)
                    f32c = rwk.tile([128, 512], F32, tag="f32c")
                    nc.scalar.copy(out=f32c[:], in_=pqk[:])
                    swp = rwk.tile([128, 512], F32, tag="swp")
                    for a in range(2):
                        nc.sync.dma_start(
                            swp[64 * a:64 * a + 32, :],
                            f32c[64 * a + 32:64 * a + 64, :],
                        )
                        nc.sync.dma_start(
                            swp[64 * a + 32:64 * a + 64, :],
                            f32c[64 * a:64 * a + 32, :],
                        )
                    nc.gpsimd.tensor_mul(f32c[:], f32c[:], cos_t[:, ssl])
                    nc.gpsimd.tensor_mul(swp[:], swp[:], sin_t[:, ssl])
                    nc.gpsimd.tensor_add(swp[:], swp[:], f32c[:])
                    for sub in range(2):
                        hh = sub  # head slot within pair
                        band = swp[64 * sub:64 * sub + 64, :]
                        if sub == 0:
                            b0 = band
                        else:
                            b0t = b0p.tile([64, 512], F32, tag="b0t")
                            nc.sync.dma_start(b0t[:], band)
                            b0 = b0t[:]
                        if qk == 0:
                            q_hl, q_hh, ql_b = tl["q_hl"], tl["q_hh"], tl["ql_b"]
                            nc.scalar.copy(out=q_hl[0:64, hh, ssl], in_=b0)
                            nc.vector.tensor_tensor(
                                q_hl[64:128, hh, ssl], b0,
                                q_hl[0:64, hh, ssl], SUB,
                            )
                            nc.sync.dma_start(
                                q_hh[0:64, hh, ssl], q_hl[0:64, hh, ssl])
                            nc.sync.dma_start(
                                q_hh[64:128, hh, ssl], q_hl[0:64, hh, ssl])
                            nc.sync.dma_start(
                                ql_b[0:64, hh, ssl], q_hl[64:128, hh, ssl])
                        else:
                            k_hl, k_hh, k_ho = tl["k_hl"], tl["k_hh"], tl["k_ho"]
                            nc.scalar.copy(out=k_hl[0:64, hh, ssl], in_=b0)
                            nc.vector.tensor_tensor(
                                k_hl[64:128, hh, ssl], b0,
                                k_hl[0:64, hh, ssl], SUB,
                            )
                            nc.sync.dma_start(
                                k_hh[0:64, hh, ssl], k_hl[0:64, hh, ssl])
                            nc.sync.dma_start(
                                k_hh[64:128, hh, ssl], k_hl[0:64, hh, ssl])
                            nc.sync.dma_start(
                                k_ho[0:64, hh, ssl], k_hl[0:64, hh, ssl])

            def cheap_gen(hh, g, tl):
                """Cheap 2-term [q,k] pass for slot (head-in-pair hh, group g):
                row maxes -> -mhat bf16 row written to ql_b[64, hh, g*512:]."""
                q_hl, k_hh, ql_b = tl["q_hl"], tl["k_hh"], tl["ql_b"]
                stage = stgp.tile([128, 128], BF16, tag="stage")
                nc.gpsimd.memset(stage[:, 4:128], 0.0)
                for qtl in range(4):
                    qt = 4 * g + qtl
                    nk = (qt + 1) * 128
                    nblk = (nk + 511) // 512
                    qsl = bass.ts(qt, 128)
                    mxp = stp.tile([128, 4], F32, tag="mxp")
                    for b in range(nblk):
                        k0 = 512 * b
                        nn = min(512, nk - k0)
                        sc = scp.tile([128, 512], F32, tag="sc")
                        nc.tensor.matmul(
                            sc[:, 0:nn],
                            q_hl[:, hh, qsl], k_hh[:, hh, bass.ds(k0, nn)],
                            start=True, stop=True,
                        )
                        if b == nblk - 1:
                            # causal mask on the diagonal 128 cols
                            # (tensor_tensor_reduce would fuse this but
                            # crashes TRN2 hw)
                            nc.vector.tensor_tensor(
                                sc[:, bass.ds(nn - 128, 128)],
                                sc[:, bass.ds(nn - 128, 128)],
                                mask512[:, 384:512], ADD,
                            )
                        nc.vector.tensor_reduce(
                            mxp[:, b:b + 1], sc[:, 0:nn],
                            mybir.AxisListType.X, MAX,
                        )
                        yield nn
                    # combine partials, negate, round to bf16
                    nc.vector.tensor_reduce(
                        stage[:, qtl:qtl + 1], mxp[:, 0:nblk],
                        mybir.AxisListType.X, MAX, negate=True,
                    )
                # transpose [q,qtl] -> [qtl,q]; write -mhat row into ql_b
                outt = stgp.tile([128, 128], BF16, tag="outt")
                nc.sync.dma_start_transpose(outt[:], stage[:])
                nc.sync.dma_start(
                    ql_b[64:65, hh, bass.ts(g, 512)], outt[0:4, 0:128])

            def stav_gen(hh, g, tl, h_abs):
                """Precise S^T + exp + AV for slot (hh, g). Yields between
                matmul groups. st leads av by 2 k-tiles."""
                k_hl, k_ho = tl["k_hl"], tl["k_ho"]
                q_hh, ql_b = tl["q_hh"], tl["ql_b"]
                jmax = 4 * g + 3
                av = avp.tile([65, 512], F32, tag="av")
                gq0 = g * 512
                pend = deque()  # (j, pts_tile, q0, nq)

                def av_mm(j, pts_t, q0, nq):
                    nc.tensor.matmul(
                        av[:, bass.ds(q0, nq)],
                        v_ext[:, j, h_abs % NHC, :], pts_t[:, 0:nq],
                        start=(j == 0), stop=(j == jmax),
                        skip_group_check=True,
                    )

                for j in range(jmax + 1):
                    q0 = max(0, (j - 4 * g) * 128)
                    nq = 512 - q0
                    jsl = bass.ts(j, 128)
                    st = scp.tile([128, 512], F32, tag="sc")
                    nc.tensor.matmul(
                        st[:, bass.ds(q0, nq)],
                        k_hl[:, hh, jsl],
                        q_hh[:, hh, bass.ds(gq0 + q0, nq)],
                        start=True, stop=False,
                    )
                    nc.tensor.matmul(
                        st[:, bass.ds(q0, nq)],
                        k_ho[0:65, hh, jsl],
                        ql_b[0:65, hh, bass.ds(gq0 + q0, nq)],
                        start=False, stop=True,
                    )
                    if j >= 4 * g:  # diagonal tile: causal mask in [k,q]
                        nc.vector.tensor_tensor(
                            st[:, bass.ds(q0, 128)],
                            st[:, bass.ds(q0, 128)], mask_t[:], ADD,
                        )
                    pts_t = ptsp.tile([128, 512], BF16, tag="pts")
                    nc.scalar.activation(
                        pts_t[:, 0:nq], st[:, bass.ds(q0, nq)],
                        EXP, scale=0.125,
                    )
                    pend.append((j, pts_t, q0, nq))
                    yield 2 * nq
                    if len(pend) > 2:
                        av_mm(*pend.popleft())
                        yield 512
                while pend:
                    av_mm(*pend.popleft())
                    yield 512
                # normalize: row 64 holds the denominator
                dro = nwk.tile([1, 512], F32, tag="dro")
                nc.vector.tensor_copy(dro[:], av[64:65, :])
                rec = nwk.tile([1, 512], F32, tag="rec")
                nc.vector.reciprocal_approx_fast(out=rec[:], in_=dro[:])
                recb = nwk.tile([64, 512], F32, tag="recb")
                nc.gpsimd.partition_broadcast(recb[:], rec[0:1, :])
                hl, pr = h_abs % 2, h_abs // 2
                nc.vector.tensor_mul(
                    av_all[64 * hl:64 * hl + 64, pr, bass.ts(g, 512)],
                    av[0:64, :], recb[:],
                )

            # ---- weave scheduler ----
            # slots in order: for p, for g, for head-in-pair
            attnq = deque()   # active attention generators [(kind, gen)]
            state = {"attn_cols": 0, "proj_cols": 0}

            def pull(gen):
                try:
                    cols = next(gen[1])
                    state["attn_cols"] += cols
                    return True
                except StopIteration:
                    try:
                        attnq.remove(gen)
                    except ValueError:
                        pass
                    return False

            def pump_attn(target_ratio=1.9, max_units=10**9):
                """Advance attention gens: primary = head of queue, weave
                with the first independent 'cheap' gen behind it."""
                units = 0
                while attnq and units < max_units and (
                    state["attn_cols"] < target_ratio * state["proj_cols"]
                    or target_ratio < 0
                ):
                    primary = attnq[0]
                    if not pull(primary):
                        continue
                    units += 1
                    sec = None
                    for gq in list(attnq)[1:]:
                        if gq[0] == "cheap":
                            sec = gq
                            break
                    if sec is not None:
                        pull(sec)
                        units += 1

            ws_cur = load_wslices(0)
            tl_cur = pair_tiles(0)
            ones_memset(tl_cur)
            xc_cur = load_xc(0)
            NCH = S // 512
            # stav generators wait one full chunk after their cheap pass so
            # the -mhat chain (DVE reduces -> XBAR -> row DMA, ~7us) is
            # always ready before the S^T mm2 reads it
            pendingA = deque()
            for p in range(NPAIR):
                ws_next = load_wslices(p + 1) if p < NPAIR - 1 else None
                tl_next = pair_tiles(p + 1) if p < NPAIR - 1 else None
                for c in range(NCH):
                    if c == NCH - 1 and tl_next is not None:
                        ones_memset(tl_next)
                    xc = xc_cur
                    if NCH * p + c < NPAIR * NCH - 1:
                        xc_cur = load_xc(c + 1)
                    for _cols in proj_gen(p, c, ws_cur, tl_cur, xc):
                        state["proj_cols"] += _cols
                        pump_attn(max_units=3)
                    g = c
                    for hh in range(2):
                        attnq.append(("cheap", cheap_gen(hh, g, tl_cur)))
                    while pendingA:
                        attnq.append(pendingA.popleft())
                    for hh in range(2):
                        pendingA.append(
                            ("stav", stav_gen(hh, g, tl_cur, 2 * p + hh)))
                    pump_attn()
                ws_cur, tl_cur = ws_next, tl_next
            # flush remaining attention
            while attnq:
                pump_attn(target_ratio=-1)
            attnq.extend(pendingA)
            pendingA.clear()
            while attnq:
                pump_attn(target_ratio=-1)

        # ---------- output projection ----------
        with tc.tile_pool(name="ops", bufs=2, space="PSUM") as ops, \
             tc.tile_pool(name="wop", bufs=1) as wop, \
             tc.tile_pool(name="owork", bufs=3) as owk:
            wo = load(wop, wo_d, "wo")
            for st in range(NQT):
                po = ops.tile([128, 2, 512], F32, tag="po")
                for half in range(2):
                    for p in range(NPAIR):
                        nc.tensor.matmul(
                            po[:, half, 0:384],
                            av_all[:, p, bass.ts(st, 128)],
                            wo[:, p, bass.ts(half, 384)],
                            start=(p == 0), stop=(p == NPAIR - 1),
                        )
                osb = owk.tile([128, D], F32, tag="osb")
                nc.scalar.copy(out=osb[:, 0:384], in_=po[:, 0, 0:384])
                nc.scalar.copy(out=osb[:, 384:768], in_=po[:, 1, 0:384])
                nc.sync.dma_start(out_d[bass.ts(st, 128), :], osb[:])

    nc.compile()
    return nc


def _rope_perm():
    p = np.zeros(DK, dtype=np.int64)
    for i in range(DK // 2):
        p[i] = 2 * i
        p[i + 32] = 2 * i + 1
    return p


def _split(a):
    hi = a.astype(bf16)
    lo = (a.astype(np.float32) - hi.astype(np.float32)).astype(bf16)
    return hi, lo


def _tile_din(a):
    # [768, F] -> [128, 6, F]
    return np.ascontiguousarray(a.reshape(DSUB, 128, -1).transpose(1, 0, 2))


def make_inputs(x, wq, wk, wv, wo, S):
    """Host-side prep: returns list of 8 in_maps (core = 2*b + g)."""
    perm = _rope_perm()
    pos = np.arange(S, dtype=np.float64)
    inv = 10000.0 ** (-2.0 * np.arange(DK // 2, dtype=np.float64) / DK)
    ang = pos[:, None] * inv[None, :]
    cosv = np.cos(ang).astype(np.float32).T  # [32, S]
    sinv = np.sin(ang).astype(np.float32).T
    cos_t = np.tile(cosv, (4, 1)).astype(np.float32)            # [128, S]
    sin_t = np.tile(
        np.concatenate([-sinv, sinv], axis=0), (2, 1)
    ).astype(np.float32)                                        # [128, S]
    # [zeros(384) | triu(-1e9, 1)]: right-aligned causal mask for the
    # cheap-max pass's last block ([q,k] orientation)
    mask512 = np.zeros((128, 512), np.float32)
    mask512[:, 384:] = np.triu(np.full((128, 128), -1e9, np.float32), 1)
    # [k,q] diagonal-tile mask: invalid k > q
    mask_t = np.tril(np.full((128, 128), -1e9, np.float32), -1)

    maps = []
    for b in range(B):
        xT = np.ascontiguousarray(x[b].T.astype(np.float32))  # [768, S]
        xh, xl = _split(xT)
        xh_t, xl_t = _tile_din(xh), _tile_din(xl)
        for g in range(2):
            hs = slice(g * CPC, (g + 1) * CPC)
            wqc = wq[hs].astype(np.float32).copy()
            wkc = wk[hs].astype(np.float32).copy()
            for arr in (wqc, wkc):
                for i in range(NHC):
                    blk = arr[i * DK:(i + 1) * DK].copy()
                    arr[i * DK:(i + 1) * DK] = blk[perm]
            wqh, wql = _split(wqc.T)  # [768, 384]
            wkh, wkl = _split(wkc.T)
            wvT = wv[hs].astype(np.float32).T.astype(bf16)
            woT = wo[:, hs].astype(np.float32).T.astype(bf16)  # [384, 768]
            maps.append({
                "xh": xh_t, "xl": xl_t,
                "wqh": _tile_din(wqh), "wql": _tile_din(wql),
                "wkh": _tile_din(wkh), "wkl": _tile_din(wkl),
                "wvT": _tile_din(wvT),
                "woT": np.ascontiguousarray(
                    woT.reshape(NPAIR, 128, D).transpose(1, 0, 2)),
                "cos_t": cos_t, "sin_t": sin_t,
                "mask512": mask512, "mask_t": mask_t,
            })
    return maps


_PROG = {}


def _prog(S):
    if S not in _PROG:
        _PROG[S] = _build(S)
    return _PROG[S]


def kernel(x, wq, wk, wv, wo, S=2048, trace=False):
    x = np.asarray(x, np.float32)
    nc = _prog(S)
    maps = make_inputs(x, np.asarray(wq), np.asarray(wk), np.asarray(wv),
                       np.asarray(wo), S)
    res = run_bass_kernel_spmd(nc, maps, list(range(8)), trace=trace)
    outs = []
    for b in range(B):
        outs.append(res.results[2 * b]["out"] + res.results[2 * b + 1]["out"])
    out = np.stack(outs)
    if trace:
        kernel.last_exec_time_ns = res.exec_time_ns
        kernel.last_results = res
    return out
